# revision 1
# baseline (speedup 1.0000x reference)
"""Causal single-head attention (B=8, S=2048, D=1024, fp32) on 8 NeuronCores.

Data-parallel over batch: one batch element per core, weights replicated.
All-bf16 SBUF-resident pipeline -- no DRAM scratch spills.
  - scores are computed as x @ (Wq Wk^T) @ x^T: M = Wq Wk^T is built once
    (65K PE-cycles) and x^T doubles as the key matrix, which deletes the
    whole K projection (131K cycles) and its 32KB/partition K^T buffer
  - x^T, Q'^T (= (xM)^T), V, P^T all held in SBUF as bf16
  - x^T via PE transposes (fp32r); Wq/Wk transposed on-chip for the M GEMM
  - diagonal score blocks at 256-wide granularity skip masked work; the
    causal mask is a -1e9 ident@mask matmul in PSUM (exp underflows to 0)
  - kd-outer "streamed" V group 0 consumes Wv slices as their DMAs land
  - PSUM evacuation biased to DVE (fast copy path); ACT does 1-in-4 + exps
"""

import numpy as np

B, S, D = 8, 2048, 1024
P = 128
NCORES = 8

_built = None


def _f32_to_bf16_bits(a):
    u = np.ascontiguousarray(a, dtype=np.float32).view(np.uint32)
    return ((u + 0x7FFF + ((u >> 16) & 1)) >> 16).astype(np.uint16)


def _build():
    import concourse.tile as tile
    import concourse.mybir as mybir
    from concourse import bacc

    FP32 = mybir.dt.float32
    FP32R = mybir.dt.float32r
    BF16 = mybir.dt.bfloat16
    AF = mybir.ActivationFunctionType

    nc = bacc.Bacc("TRN2", target_bir_lowering=False, debug=False, num_devices=NCORES)
    x_d = nc.dram_tensor("x", [S, D], FP32R, kind="ExternalInput").ap()
    wq_d = nc.dram_tensor("Wq", [D, D], FP32, kind="ExternalInput").ap()
    wk_d = nc.dram_tensor("Wk", [D, D], FP32, kind="ExternalInput").ap()
    wv_d = nc.dram_tensor("Wv", [D, D], FP32, kind="ExternalInput").ap()
    out_d = nc.dram_tensor("out", [S, D], FP32, kind="ExternalOutput").ap()

    ident_bf_c = nc.inline_tensor(
        _f32_to_bf16_bits(np.eye(P, dtype=np.float32)), name="ident_bf_c"
    )
    ident_c = nc.inline_tensor(np.eye(P, dtype=np.float32), name="ident_c")
    # masks: cols 0..255 diag pattern d=0, cols 256..511 d=128, cols 512..513 ones
    yy = np.arange(256)[None, :]
    pp = np.arange(P)[:, None]
    m0 = np.where(yy >= pp, 0.0, -1e9).astype(np.float32)
    m1 = np.where(yy >= pp + 128, 0.0, -1e9).astype(np.float32)
    masks_np = np.concatenate([m0, m1, np.ones((P, 2), np.float32)], axis=1)
    masks_bf_c = nc.inline_tensor(_f32_to_bf16_bits(masks_np), name="masks_bf_c")

    with tile.TileContext(nc) as tc:
        with (
            tc.tile_pool(name="smalls", bufs=1) as smalls,
            tc.tile_pool(name="xst", bufs=8) as xst,
            tc.tile_pool(name="xtp", bufs=1) as xtp,
            tc.tile_pool(name="wstage", bufs=2) as wstage,
            tc.tile_pool(name="wp", bufs=3) as wp,
            tc.tile_pool(name="vp", bufs=1) as vp,
            tc.tile_pool(name="qtp", bufs=2) as qtp,
            tc.tile_pool(name="e5", bufs=13) as e5,
            tc.tile_pool(name="e2", bufs=8) as e2,
            tc.tile_pool(name="osb", bufs=4) as osb,
            tc.tile_pool(name="rcp", bufs=4) as rcp,
            tc.tile_pool(name="ps", bufs=8, space="PSUM") as ps,
        ):
            def cpy(use_act, out, in_):
                if use_act:
                    nc.scalar.copy(out=out, in_=in_)
                else:
                    nc.vector.tensor_copy(out=out, in_=in_)

            ident_bf = smalls.tile([P, P], BF16, tag="ident_bf")
            nc.sync.dma_start(out=ident_bf, in_=ident_bf_c.ap().bitcast(BF16))
            masks = smalls.tile([P, 514], BF16, tag="masks")
            nc.sync.dma_start(out=masks, in_=masks_bf_c.ap().bitcast(BF16))
            ident = smalls.tile([P, P], FP32R, tag="ident")
            nc.gpsimd.dma_start(out=ident, in_=ident_c.ap().bitcast(FP32R))
            ones = masks[:, 512:514]

            # HAM warm-up on the constants while the x/W preamble DMAs stream.
            for r in range(4):
                dps = ps.tile([P, 512], FP32, tag="ps")
                for j in range(4):
                    nc.tensor.matmul(
                        dps, lhsT=ident_bf, rhs=masks[:, 0:512],
                        start=(j == 0), stop=(j == 3),
                    )

            x_pend = {}

            def load_x(si):
                x_f = xst.tile([P, D], FP32R, tag="xst")
                (nc.sync, nc.gpsimd)[si % 2].dma_start(
                    out=x_f, in_=x_d[si * P:(si + 1) * P, :]
                )
                x_pend[si] = x_f

            def load_w(w_d, w_bf):
                # DMA fp32 slices; cast to bf16 on DVE (fast copy path).
                for kd in range(8):
                    w_t = wstage.tile([P, D], FP32, tag="wstage")
                    (nc.sync, nc.gpsimd)[kd % 2].dma_start(
                        out=w_t, in_=w_d[kd * P:(kd + 1) * P, :]
                    )
                    nc.vector.tensor_copy(out=w_bf[:, kd, :], in_=w_t)

            xt = xtp.tile([P, 8, S], BF16, tag="xt")
            wv_sb = wp.tile([P, 8, D], BF16, tag="wp")
            v_sb = vp.tile([P, 16, D], BF16, tag="v")

            for si in range(8):
                load_x(si)
            load_w(wv_d, wv_sb)

            TAILS = {0: (8, 9), 1: (10, 11, 12, 13), 2: (14, 15), 3: ()}

            def transpose_group(g):
                xts = [x_pend.pop(si) for si in range(4 * g, 4 * g + 4)]
                for kd in range(8):
                    tp4 = ps.tile([P, 512], FP32R, tag="ps")
                    for j in range(4):
                        nc.tensor.matmul(
                            tp4[:, j * P:(j + 1) * P],
                            lhsT=xts[j][:, kd * P:(kd + 1) * P],
                            rhs=ident,
                            is_transpose=True,
                            start=(j == 0),
                            stop=(j == 3),
                        )
                    cpy(kd % 4 == 1, xt[:, kd, g * 512:(g + 1) * 512], tp4)
                for si in TAILS[g]:
                    load_x(si)

            def ps_tile():
                t = ps.tile([P, 512], FP32, tag="ps")
                return t

            def v_proj_group_streamed(g):
                # kd-outer over 4 concurrent PSUM tiles: consumes Wv slice kd
                # as its DMA+cast lands instead of needing full Wv up front.
                for ec in range(2):
                    psts = [ps_tile() for _ in range(4)]
                    for kd in range(8):
                        for sj in range(4):
                            st_i = 4 * g + sj
                            nc.tensor.matmul(
                                psts[sj],
                                lhsT=xt[:, kd, st_i * P:(st_i + 1) * P],
                                rhs=wv_sb[:, kd, ec * 512:(ec + 1) * 512],
                                start=(kd == 0),
                                stop=(kd == 7),
                            )
                    for sj in range(4):
                        st_i = 4 * g + sj
                        cpy(sj == 1, v_sb[:, st_i, ec * 512:(ec + 1) * 512],
                            psts[sj])

            def v_proj_group(g):
                for st_i in range(4 * g, 4 * g + 4):
                    for ec in range(2):
                        pst = ps.tile([P, 512], FP32, tag="ps")
                        for kd in range(8):
                            nc.tensor.matmul(
                                pst,
                                lhsT=xt[:, kd, st_i * P:(st_i + 1) * P],
                                rhs=wv_sb[:, kd, ec * 512:(ec + 1) * 512],
                                start=(kd == 0),
                                stop=(kd == 7),
                            )
                        cpy((st_i + ec) % 4 == 1,
                            v_sb[:, st_i, ec * 512:(ec + 1) * 512], pst)

            qt_tiles = {}
            est5 = {}
            est2 = {}

            def w_transpose(w_sb, wt_sb):
                # wt_sb[p, et, kd*128+j] = W[kd*128+j, et*128+p]
                for et in range(8):
                    for g2 in range(2):
                        tpw = ps.tile([P, 512], BF16, tag="ps")
                        for j in range(4):
                            nc.tensor.matmul(
                                tpw[:, j * P:(j + 1) * P],
                                lhsT=w_sb[:, 4 * g2 + j, et * P:(et + 1) * P],
                                rhs=ident_bf,
                                is_transpose=True,
                                start=(j == 0),
                                stop=(j == 3),
                            )
                        cpy((et + g2) % 4 == 1,
                            wt_sb[:, et, g2 * 512:(g2 + 1) * 512], tpw)

            def m_compute(wqt_sb, wkt_sb, m_sb):
                # M[d, d'] = sum_e Wq[d, e] * Wk[d', e]  (= Wq @ Wk^T)
                for dt in range(8):
                    for cp in range(2):
                        pst = ps.tile([P, 512], FP32, tag="ps")
                        for et in range(8):
                            nc.tensor.matmul(
                                pst,
                                lhsT=wqt_sb[:, et, dt * P:(dt + 1) * P],
                                rhs=wkt_sb[:, et, cp * 512:(cp + 1) * 512],
                                start=(et == 0),
                                stop=(et == 7),
                            )
                        cpy((dt + cp) % 4 == 1,
                            m_sb[:, dt, cp * 512:(cp + 1) * 512], pst)

            def q_proj(c, w_sb):
                qt_sb = qtp.tile([P, 8, 512], BF16, tag="qt")
                qt_tiles[c] = qt_sb
                for et in range(8):
                    pst = ps.tile([P, 512], FP32, tag="ps")
                    for kd in range(8):
                        nc.tensor.matmul(
                            pst,
                            lhsT=w_sb[:, kd, et * P:(et + 1) * P],
                            rhs=xt[:, kd, c * 512:(c + 1) * 512],
                            start=(kd == 0),
                            stop=(kd == 7),
                        )
                    cpy(et % 4 == 1, qt_sb[:, et, :], pst)

            def offdiag_scores(c):
                qt_sb = qt_tiles[c]
                for k in range(4 * c):
                    sps = ps.tile([P, 512], FP32, tag="ps")
                    for e in range(8):
                        nc.tensor.matmul(
                            sps,
                            lhsT=xt[:, e, k * P:(k + 1) * P],
                            rhs=qt_sb[:, e, :],
                            start=(e == 0),
                            stop=(e == 7),
                        )
                    est = e5.tile([P, 512], BF16, tag="e5")
                    nc.scalar.activation(out=est, in_=sps, func=AF.Exp, scale=0.03125)
                    est5[(c, k)] = est

            # diag groups in consumption order; mask col offset in `masks` or None
            DIAG = [
                (0, 0, 0), (1, 0, 256), (0, 1, None),
                (1, 1, None), (2, 1, 0), (3, 1, 256),
            ]

            def diag_scores(c):
                qt_sb = qt_tiles[c]
                for (j, h, moff) in DIAG:
                    k = 4 * c + j
                    sps = ps.tile([P, 256], FP32, tag="ps")
                    for e in range(8):
                        nc.tensor.matmul(
                            sps,
                            lhsT=xt[:, e, k * P:(k + 1) * P],
                            rhs=qt_sb[:, e, h * 256:(h + 1) * 256],
                            start=(e == 0),
                            stop=(e == 7 and moff is None),
                        )
                    if moff is not None:
                        nc.tensor.matmul(
                            sps,
                            lhsT=ident_bf,
                            rhs=masks[:, moff:moff + 256],
                            start=False,
                            stop=True,
                        )
                    est = e2.tile([P, 256], BF16, tag="e2")
                    nc.scalar.activation(out=est, in_=sps, func=AF.Exp, scale=0.03125)
                    est2[(c, j, h)] = est

            def av_stage(c):
                for i in range(4):
                    q_abs = 4 * c + i
                    h = i // 2

                    def p_slice(k_abs):
                        if k_abs < 4 * c:
                            return est5[(c, k_abs)][:, i * P:(i + 1) * P]
                        j = k_abs - 4 * c
                        return est2[(c, j, h)][:, (i - 2 * h) * P:(i - 2 * h + 1) * P]

                    rs = ps.tile([P, 2], FP32, tag="ps")
                    for k_abs in range(q_abs + 1):
                        nc.tensor.matmul(
                            rs, lhsT=p_slice(k_abs), rhs=ones,
                            start=(k_abs == 0), stop=(k_abs == q_abs),
                        )
                    rec = rcp.tile([P, 1], FP32, tag="rcp")
                    nc.vector.reciprocal(rec, rs[:, 0:1])
                    for h2 in range(2):
                        ops_t = ps.tile([P, 512], FP32, tag="ps")
                        for k_abs in range(q_abs + 1):
                            nc.tensor.matmul(
                                ops_t,
                                lhsT=p_slice(k_abs),
                                rhs=v_sb[:, k_abs, h2 * 512:(h2 + 1) * 512],
                                start=(k_abs == 0),
                                stop=(k_abs == q_abs),
                            )
                        o_sb = osb.tile([P, 512], FP32, tag="osb")
                        nc.vector.tensor_scalar_mul(o_sb, ops_t, rec)
                        (nc.sync, nc.gpsimd)[(q_abs + h2) % 2].dma_start(
                            out=out_d[q_abs * P:(q_abs + 1) * P,
                                      h2 * 512:(h2 + 1) * 512],
                            in_=o_sb,
                        )

            # ---- phase A: transposes interleaved with V projection ----
            transpose_group(0)
            transpose_group(1)
            v_proj_group_streamed(0)
            v_proj_group(1)
            transpose_group(2)
            v_proj_group(2)
            transpose_group(3)
            wq_sb = wp.tile([P, 8, D], BF16, tag="wp")
            load_w(wq_d, wq_sb)
            v_proj_group(3)
            wk_sb = wp.tile([P, 8, D], BF16, tag="wp")
            load_w(wk_d, wk_sb)

            # ---- W transposes + M = Wq @ Wk^T ----
            wqt_sb = wp.tile([P, 8, D], BF16, tag="wp")
            w_transpose(wq_sb, wqt_sb)
            wkt_sb = wp.tile([P, 8, D], BF16, tag="wp")
            w_transpose(wk_sb, wkt_sb)
            m_sb = wp.tile([P, 8, D], BF16, tag="wp")
            m_compute(wqt_sb, wkt_sb, m_sb)

            # ---- phase B: per-chunk Q' proj (x@M), scores vs x^T, AV ----
            q_proj(0, m_sb)
            q_proj(1, m_sb)
            diag_scores(0)
            offdiag_scores(1)
            diag_scores(1)
            av_stage(0)
            q_proj(2, m_sb)
            av_stage(1)
            offdiag_scores(2)
            diag_scores(2)
            q_proj(3, m_sb)
            av_stage(2)
            offdiag_scores(3)
            diag_scores(3)
            av_stage(3)

    nc.compile()
    return nc


def _get_nc():
    global _built
    if _built is None:
        _built = _build()
    return _built


def _run(inputs, trace=False):
    from concourse.bass_utils import run_bass_kernel_spmd

    x = inputs["x"]
    in_maps = [
        {
            "x": np.ascontiguousarray(x[c], dtype=np.float32),
            "Wq": np.asarray(inputs["Wq"], dtype=np.float32),
            "Wk": np.asarray(inputs["Wk"], dtype=np.float32),
            "Wv": np.asarray(inputs["Wv"], dtype=np.float32),
        }
        for c in range(NCORES)
    ]
    res = run_bass_kernel_spmd(
        nc=_get_nc(), in_maps=in_maps, core_ids=list(range(NCORES)), trace=trace
    )
    out = np.stack([res.results[c]["out"] for c in range(NCORES)], axis=0)
    return out, res


def kernel(x, Wq, Wk, Wv):
    out, _ = _run({"x": x, "Wq": Wq, "Wk": Wk, "Wv": Wv}, trace=False)
    return out



# revision 12
# speedup vs baseline: 1.1710x; 1.1710x over previous
"""Causal single-head attention (B=8, S=2048, D=1024, fp32) on 8 NeuronCores.

Data-parallel over batch: one batch element per core, weights replicated.
fp8(e4m3) DoubleRow pipeline: every GEMM operand is split hi/lo
(A ~= A_hi + A_lo, both e4m3) and products computed as the 3-term sum
A_hi@B_hi + A_hi@B_lo + A_lo@B_hi on the PE's DoubleRow path (2 k-tiles
per instruction at 0.5 cycles/row) -- 0.75x the bf16 PE time with
bf16-class accuracy (measured 2.7e-3 rms vs fp64).
  - scores via M-trick: M = (32Wq)(32Wk)^T computed once, rescaled 2^-6
    at its hi/lo split so Q'' = x@M''' stays in e4m3 range (sigma 16)
  - x^T via PE fp32r transposes; Wq/Wk transposed on-chip in fp16
  - Wk/Wv/Wq scaled x32 before splitting so the lo residuals clear the
    e4m3 subnormal flush threshold (2^-10); V' = 32V descaled through
    the rowsum (ones value 32)
  - exp on ACT with scale 1/512 and bias -0.5 (keeps est <= ~150 < 240
    e4m3 max); est split hi/lo on Pool from an fp16 staging tile
  - causal diag at 512/384/256/128 tapered widths, mask as the group's
    leading bf16 ident@mask matmul
  - engine split: PSUM-sourced hi on ACT, all PSUM lo on DVE (Pool
    cannot read PSUM), SBUF-sourced est/W work on Pool
"""

import numpy as np

B, S, D = 8, 2048, 1024
P = 128
NCORES = 8

_built = None


def _f32_to_bf16_bits(a):
    u = np.ascontiguousarray(a, dtype=np.float32).view(np.uint32)
    return ((u + 0x7FFF + ((u >> 16) & 1)) >> 16).astype(np.uint16)


def _f32_to_f16_bits(a):
    return np.ascontiguousarray(a, dtype=np.float32).astype(np.float16).view(np.uint16)


def _build():
    import concourse.tile as tile
    import concourse.mybir as mybir
    from concourse import bacc

    FP32 = mybir.dt.float32
    FP32R = mybir.dt.float32r
    FP16 = mybir.dt.float16
    BF16 = mybir.dt.bfloat16
    FP8 = mybir.dt.float8e4
    AF = mybir.ActivationFunctionType
    ALU = mybir.AluOpType
    DR = mybir.MatmulPerfMode.DoubleRow

    SSCALE = 1.0 / 512.0  # scores psum = 512 * true scaled scores
    EBIAS = -0.5
    MSCALE = 2.0 ** -6    # M psum (=1024*M) -> m8 (=16*M)

    nc = bacc.Bacc("TRN2", target_bir_lowering=False, debug=False, num_devices=NCORES)
    x_d = nc.dram_tensor("x", [S, D], FP32R, kind="ExternalInput").ap()
    wq_d = nc.dram_tensor("Wq", [D, D], FP32R, kind="ExternalInput").ap()
    wk_d = nc.dram_tensor("Wk", [D, D], FP32R, kind="ExternalInput").ap()
    wv_d = nc.dram_tensor("Wv", [D, D], FP32R, kind="ExternalInput").ap()
    out_d = nc.dram_tensor("out", [S, D], BF16, kind="ExternalOutput").ap()

    ident_c = nc.inline_tensor(np.eye(P, dtype=np.float32), name="ident_c")
    ident_bf_c = nc.inline_tensor(
        _f32_to_bf16_bits(np.eye(P, dtype=np.float32)), name="ident_bf_c"
    )
    ident16_c = nc.inline_tensor(
        _f32_to_f16_bits(np.eye(P, dtype=np.float32)), name="ident16_c"
    )
    # diag mask: cols 0..127 are the diagonal 128-block (-1e9 where q<k),
    # cols 128..511 zero (strictly-lower blocks, unmasked)
    yy = np.arange(P)[None, :]
    pp = np.arange(P)[:, None]
    m_np = np.where(yy < pp, -1e9, 0.0).astype(np.float32)
    maskd_c = nc.inline_tensor(_f32_to_bf16_bits(m_np), name="maskd_c")

    with tile.TileContext(nc) as tc:
        with (
            tc.tile_pool(name="smalls", bufs=1) as smalls,
            tc.tile_pool(name="xst", bufs=5) as xst,
            tc.tile_pool(name="xtp", bufs=1) as xtp,
            tc.tile_pool(name="wst", bufs=3) as wst,
            tc.tile_pool(name="w16st", bufs=5) as w16st,
            tc.tile_pool(name="w8pool", bufs=3) as w8pool,
            tc.tile_pool(name="m8p", bufs=1) as m8p,
            tc.tile_pool(name="v8p", bufs=1) as v8p,
            tc.tile_pool(name="qt8p", bufs=1) as qt8p,
            tc.tile_pool(name="estp", bufs=6) as estp,
            tc.tile_pool(name="estA", bufs=1) as estAp,
            tc.tile_pool(name="estB", bufs=1) as estBp,
            tc.tile_pool(name="t16p", bufs=2) as t16p,
            tc.tile_pool(name="osb", bufs=2) as osbp,
            tc.tile_pool(name="rcp", bufs=2) as rcpp,
            tc.tile_pool(name="ps", bufs=8, space="PSUM") as ps,
        ):
            ident = smalls.tile([P, P], FP32R, tag="ident")
            nc.sync.dma_start(out=ident, in_=ident_c.ap().bitcast(FP32R))
            ident_bf = smalls.tile([P, P], BF16, tag="ident_bf")
            nc.sync.dma_start(out=ident_bf, in_=ident_bf_c.ap().bitcast(BF16))
            ident16 = smalls.tile([P, P], FP16, tag="ident16")
            nc.sync.dma_start(out=ident16, in_=ident16_c.ap().bitcast(FP16))
            maskd = smalls.tile([P, P], BF16, tag="maskd")
            nc.sync.dma_start(out=maskd, in_=maskd_c.ap().bitcast(BF16))
            ones8 = smalls.tile([P, 2, 2], FP8, tag="ones8")
            nc.gpsimd.memset(ones8, 32.0)
            bias_t = smalls.tile([P, 1], FP32, tag="bias_t")
            nc.gpsimd.memset(bias_t, EBIAS)

            # HAM warm-up on the constants while the preamble DMAs stream.
            for r in range(4):
                dps = ps.tile([P, 512], FP32, tag="ps")
                for j in range(4):
                    nc.tensor.matmul(
                        dps[:, j * P:(j + 1) * P], lhsT=ident_bf,
                        rhs=maskd, start=(j == 0), stop=(j == 3),
                    )

            x_pend = {}

            def load_x(si):
                x_f = xst.tile([P, D], FP32R, tag="xst")
                nc.sync.dma_start(out=x_f, in_=x_d[si * P:(si + 1) * P, :])
                x_pend[si] = x_f

            # x^T pairs: [p, dpair, dslot, hl, s]
            xt8 = xtp.tile([P, 4, 2, 2, S], FP8, tag="xt8")

            def transpose_group(g, tails=()):
                xts = [x_pend.pop(si) for si in range(4 * g, 4 * g + 4)]
                for kd in range(8):
                    tp4 = ps.tile([P, 512], FP32R, tag="ps")
                    for j in range(4):
                        nc.tensor.matmul(
                            tp4[:, j * P:(j + 1) * P],
                            lhsT=xts[j][:, kd * P:(kd + 1) * P],
                            rhs=ident, is_transpose=True,
                            start=(j == 0), stop=(j == 3),
                        )
                    hi = xt8[:, kd // 2, kd % 2, 0, g * 512:(g + 1) * 512]
                    nc.scalar.activation(out=hi, in_=tp4, func=AF.Copy)
                    nc.vector.tensor_tensor(
                        out=xt8[:, kd // 2, kd % 2, 1, g * 512:(g + 1) * 512],
                        in0=tp4, in1=hi, op=ALU.subtract,
                    )
                for fn in tails:
                    fn()

            wv8 = w8pool.tile([P, 4, 2, 2, D], FP8, tag="w8")

            def load_wv_chunk(kd):
                w_t = wst.tile([P, D], FP32R, tag="wst")
                nc.sync.dma_start(out=w_t, in_=wv_d[kd * P:(kd + 1) * P, :])
                hi = wv8[:, kd // 2, kd % 2, 0, :]
                nc.gpsimd.tensor_scalar_mul(hi, w_t, 32.0)
                nc.vector.scalar_tensor_tensor(
                    out=wv8[:, kd // 2, kd % 2, 1, :], in0=w_t, scalar=32.0,
                    in1=hi, op0=ALU.mult, op1=ALU.subtract,
                )

            def load_w_half(w_d, g2):
                chunks = []
                for kd in range(4 * g2, 4 * g2 + 4):
                    w_t = wst.tile([P, D], FP32R, tag="wst")
                    nc.sync.dma_start(out=w_t, in_=w_d[kd * P:(kd + 1) * P, :])
                    w16 = w16st.tile([P, D], FP16, tag="w16")
                    nc.gpsimd.tensor_scalar_mul(w16, w_t, 32.0)
                    chunks.append(w16)
                return chunks

            v8 = v8p.tile([P, 8, 2, 2, D], FP8, tag="v8")

            def mm3(pst, lhs_of, rhs_of, npair, extra_first=None):
                """12-instr 3-term DoubleRow group accumulating into pst.

                lhs_of(t, hl) / rhs_of(t, hl) give [P, 2, F] APs for pair t.
                extra_first: (lhsT, rhs) bf16 matmul issued first (mask).
                """
                n = 0
                total = 3 * npair + (1 if extra_first is not None else 0)
                for t in range(npair):
                    for (sa, sb) in ((0, 0), (0, 1), (1, 0)):
                        nc.tensor.matmul(
                            pst, lhsT=lhs_of(t, sa), rhs=rhs_of(t, sb),
                            start=(n == 0), stop=(n == total - 1), perf_mode=DR,
                        )
                        n += 1
                        if n == 1 and extra_first is not None:
                            lt, rt = extra_first
                            nc.tensor.matmul(pst[:, 0:P], lhsT=lt, rhs=rt,
                                             start=False, stop=False)
                            n += 1

            def ps_tile(shape=None, dtype=None):
                t = ps.tile(shape or [P, 512], dtype or FP32, tag="ps")
                return t

            def v_proj_group(g, after_kp=None, ecs=(0, 1)):
                # kp-outer over 4 concurrent PSUM tiles per e-chunk: consumes
                # Wv pairs as their DMAs+splits land. after_kp[(ec, kp)] emits
                # filler work (e.g. a W transpose half) mid-group.
                for ec in ecs:
                    psts = [ps_tile() for _ in range(4)]
                    for kp in range(4):
                        if after_kp and (ec, kp) in after_kp:
                            after_kp[(ec, kp)]()
                        for sj in range(4):
                            st_i = 4 * g + sj
                            for ni, (sa, sb) in enumerate(((0, 0), (0, 1), (1, 0))):
                                nc.tensor.matmul(
                                    psts[sj],
                                    lhsT=xt8[:, kp, :, sa, st_i * P:(st_i + 1) * P],
                                    rhs=wv8[:, kp, :, sb, ec * 512:(ec + 1) * 512],
                                    start=(kp == 0 and ni == 0),
                                    stop=(kp == 3 and ni == 2),
                                    perf_mode=DR,
                                )
                    for sj in range(4):
                        st_i = 4 * g + sj
                        hi = v8[:, st_i // 2, st_i % 2, 0, ec * 512:(ec + 1) * 512]
                        nc.scalar.activation(out=hi, in_=psts[sj], func=AF.Copy)
                        nc.vector.tensor_tensor(
                            out=v8[:, st_i // 2, st_i % 2, 1, ec * 512:(ec + 1) * 512],
                            in0=psts[sj], in1=hi, op=ALU.subtract,
                        )

            def w_transpose_half(chunks, wt8, g2):
                # wt8[p, epair, eslot, hl, d] = 32*W[d, e]^T for d-half g2,
                # fp16 transposes of the Pool-cast (x32) staged chunks.
                for et in range(8):
                    tpw = ps_tile([P, 512], FP16)
                    for j in range(4):
                        nc.tensor.matmul(
                            tpw[:, j * P:(j + 1) * P],
                            lhsT=chunks[j][:, et * P:(et + 1) * P],
                            rhs=ident16, is_transpose=True,
                            start=(j == 0), stop=(j == 3),
                        )
                    hi = wt8[:, et // 2, et % 2, 0, g2 * 512:(g2 + 1) * 512]
                    nc.scalar.activation(out=hi, in_=tpw, func=AF.Copy)
                    nc.vector.tensor_tensor(
                        out=wt8[:, et // 2, et % 2, 1, g2 * 512:(g2 + 1) * 512],
                        in0=tpw, in1=hi, op=ALU.subtract,
                    )

            def m_compute(wqt8, wkt8, m8):
                # m8 = 2^-6 * (32Wq)(32Wk)^T = 16*M, laid out [p, dpair,
                # dslot, hl, e'] as the Q'-projection lhsT.
                for cp in range(2):
                    for dt in range(8):
                        pst = ps.tile([P, 512], FP32, tag="ps")
                        mm3(
                            pst,
                            lambda t, hl: wqt8[:, t, :, hl, dt * P:(dt + 1) * P],
                            lambda t, hl: wkt8[:, t, :, hl, cp * 512:(cp + 1) * 512],
                            4,
                        )
                        hi = m8[:, dt // 2, dt % 2, 0, cp * 512:(cp + 1) * 512]
                        nc.scalar.activation(out=hi, in_=pst, func=AF.Copy, scale=MSCALE)
                        nc.vector.scalar_tensor_tensor(
                            out=m8[:, dt // 2, dt % 2, 1, cp * 512:(cp + 1) * 512],
                            in0=pst, scalar=MSCALE, in1=hi,
                            op0=ALU.mult, op1=ALU.subtract,
                        )

            qt_tiles = {}

            def q_proj(c, m8):
                qt8 = qt8p.tile([P, 4, 2, 2, 512], FP8, tag="qt8")
                qt_tiles[c] = qt8
                for et in range(8):
                    pst = ps.tile([P, 512], FP32, tag="ps")
                    mm3(
                        pst,
                        lambda t, hl: m8[:, t, :, hl, et * P:(et + 1) * P],
                        lambda t, hl: xt8[:, t, :, hl, c * 512:(c + 1) * 512],
                        4,
                    )
                    hi = qt8[:, et // 2, et % 2, 0, :]
                    nc.scalar.activation(out=hi, in_=pst, func=AF.Copy)
                    nc.vector.tensor_tensor(
                        out=qt8[:, et // 2, et % 2, 1, :],
                        in0=pst, in1=hi, op=ALU.subtract,
                    )

            est_off = {}
            est_dA = {}
            est_dB = {}

            est_ctr = [0]

            def est_split(pst, w, dest_of):
                """exp(psum) -> fp16 staging -> hi/lo fp8, alternating the
                hi/lo engine split between DVE and Pool to halve the serial
                per-tile latency (Pool alone can't keep up with PE)."""
                t16 = t16p.tile([P, 512], FP16, tag="t16")
                nc.scalar.activation(
                    out=t16[:, 0:w], in_=pst, func=AF.Exp, bias=bias_t, scale=SSCALE
                )
                hi = dest_of(0)
                est_ctr[0] += 1
                if est_ctr[0] % 2 == 0:
                    nc.vector.tensor_copy(out=hi, in_=t16[:, 0:w])
                    nc.gpsimd.tensor_tensor(
                        out=dest_of(1), in0=t16[:, 0:w], in1=hi, op=ALU.subtract
                    )
                else:
                    nc.gpsimd.tensor_copy(out=hi, in_=t16[:, 0:w])
                    nc.vector.tensor_tensor(
                        out=dest_of(1), in0=t16[:, 0:w], in1=hi, op=ALU.subtract
                    )

            def offdiag_scores(c):
                qt8 = qt_tiles[c]
                for k in range(4 * c):
                    if k % 2 == 0:
                        eo = estp.tile([P, 2, 2, 512], FP8, tag="est")
                        est_off[(c, k // 2)] = eo
                    eo = est_off[(c, k // 2)]
                    pst = ps.tile([P, 512], FP32, tag="ps")
                    mm3(
                        pst,
                        lambda t, hl: xt8[:, t, :, hl, k * P:(k + 1) * P],
                        lambda t, hl: qt8[:, t, :, hl, :],
                        4,
                    )
                    est_split(pst, 512, lambda hl, eo=eo, k=k: eo[:, k % 2, hl, :])

            def diag_scores(c):
                qt8 = qt_tiles[c]
                eA = estAp.tile([P, 2, 2, 512], FP8, tag="estA")
                eB = estBp.tile([P, 2, 2, 256], FP8, tag="estB")
                est_dA[c] = eA
                est_dB[c] = eB
                nc.gpsimd.memset(eA[:, 1, :, 0:128], 0.0)
                nc.gpsimd.memset(eB[:, 1, :, 0:128], 0.0)
                for j in range(4):
                    w = 512 - 128 * j
                    k = 4 * c + j
                    pst = ps.tile([P, w], FP32, tag="ps")
                    mm3(
                        pst,
                        lambda t, hl: xt8[:, t, :, hl, k * P:(k + 1) * P],
                        lambda t, hl, j=j, w=w: qt8[:, t, :, hl, j * 128:512],
                        4,
                        extra_first=(ident_bf, maskd),
                    )
                    if j == 0:
                        dest = lambda hl: eA[:, 0, hl, 0:512]
                    elif j == 1:
                        dest = lambda hl: eA[:, 1, hl, 128:512]
                    elif j == 2:
                        dest = lambda hl: eB[:, 0, hl, 0:256]
                    else:
                        dest = lambda hl: eB[:, 1, hl, 128:256]
                    est_split(pst, w, dest)

            def av_stage(c):
                for i in range(4):
                    q_abs = 4 * c + i
                    # (est tile, column offset for q-tile i, v8 pair index)
                    pairs = [(est_off[(c, t)], i * P, t) for t in range(2 * c)]
                    pairs.append((est_dA[c], i * P, 2 * c))
                    if i >= 2:
                        pairs.append((est_dB[c], (i - 2) * P, 2 * c + 1))

                    rs = ps.tile([P, 2], FP32, tag="ps")
                    n = 0
                    tot = 2 * len(pairs)
                    for (eo, off, _kp) in pairs:
                        for hl in range(2):
                            nc.tensor.matmul(
                                rs, lhsT=eo[:, :, hl, off:off + P], rhs=ones8,
                                start=(n == 0), stop=(n == tot - 1), perf_mode=DR,
                            )
                            n += 1
                    rec = rcpp.tile([P, 1], FP32, tag="rcp")
                    nc.vector.reciprocal(rec, rs[:, 0:1])

                    for h2 in range(2):
                        opst = ps.tile([P, 512], FP32, tag="ps")
                        n = 0
                        tot = 3 * len(pairs)
                        for (eo, off, kp) in pairs:
                            for (sa, sb) in ((0, 0), (0, 1), (1, 0)):
                                nc.tensor.matmul(
                                    opst,
                                    lhsT=eo[:, :, sa, off:off + P],
                                    rhs=v8[:, kp, :, sb, h2 * 512:(h2 + 1) * 512],
                                    start=(n == 0), stop=(n == tot - 1),
                                    perf_mode=DR,
                                )
                                n += 1
                        o_sb = osbp.tile([P, 512], BF16, tag="osb")
                        nc.scalar.activation(out=o_sb, in_=opst, func=AF.Copy, scale=rec)
                        nc.sync.dma_start(
                            out=out_d[q_abs * P:(q_abs + 1) * P,
                                      h2 * 512:(h2 + 1) * 512],
                            in_=o_sb,
                        )

            # ---- phase A: x ingest/transposes + V proj, W loads ----
            for si in range(8):
                load_x(si)
            transpose_group(0, tails=[
                lambda: load_wv_chunk(0), lambda: load_wv_chunk(1),
                lambda: load_x(8), lambda: load_x(9)])
            transpose_group(1, tails=[
                lambda: load_wv_chunk(2), lambda: load_wv_chunk(3),
                lambda: load_x(10), lambda: load_x(11),
                lambda: load_wv_chunk(4), lambda: load_wv_chunk(5),
                lambda: load_x(12), lambda: load_x(13),
                lambda: load_wv_chunk(6), lambda: load_wv_chunk(7)])
            v_proj_group(0)
            transpose_group(2, tails=[
                lambda: load_x(14), lambda: load_x(15)])
            v_proj_group(1)
            wq_h0 = load_w_half(wq_d, 0)
            transpose_group(3)
            wq_h1 = load_w_half(wq_d, 1)
            wqt8 = w8pool.tile([P, 4, 2, 2, D], FP8, tag="w8")
            wkt8 = w8pool.tile([P, 4, 2, 2, D], FP8, tag="w8")
            wk_holder = []

            # W transposes interleave with v2 so PE tracks the Wq/Wk DMAs;
            # M follows immediately; v3 fills the m8/qt split latencies.
            w_transpose_half(wq_h0, wqt8, 0)
            wk_holder.append(load_w_half(wk_d, 0))
            w_transpose_half(wq_h1, wqt8, 1)

            def _wtk0():
                w_transpose_half(wk_holder[0], wkt8, 0)
                wk_holder.append(load_w_half(wk_d, 1))

            v_proj_group(2, after_kp={(1, 0): _wtk0})
            w_transpose_half(wk_holder[1], wkt8, 1)
            m8 = m8p.tile([P, 4, 2, 2, D], FP8, tag="m8")
            m_compute(wqt8, wkt8, m8)

            # ---- phase B: per-chunk Q'' proj, scores vs x^T, AV ----
            q_proj(0, m8)
            v_proj_group(3, ecs=(0,))
            diag_scores(0)
            v_proj_group(3, ecs=(1,))
            q_proj(1, m8)
            offdiag_scores(1)
            diag_scores(1)
            av_stage(0)
            q_proj(2, m8)
            av_stage(1)
            offdiag_scores(2)
            diag_scores(2)
            q_proj(3, m8)
            av_stage(2)
            diag_scores(3)
            offdiag_scores(3)
            av_stage(3)

    nc.compile()
    return nc


def _get_nc():
    global _built
    if _built is None:
        _built = _build()
    return _built


def _run(inputs, trace=False):
    from concourse.bass_utils import run_bass_kernel_spmd

    x = inputs["x"]
    in_maps = [
        {
            "x": np.ascontiguousarray(x[c], dtype=np.float32),
            "Wq": np.asarray(inputs["Wq"], dtype=np.float32),
            "Wk": np.asarray(inputs["Wk"], dtype=np.float32),
            "Wv": np.asarray(inputs["Wv"], dtype=np.float32),
        }
        for c in range(NCORES)
    ]
    res = run_bass_kernel_spmd(
        nc=_get_nc(), in_maps=in_maps, core_ids=list(range(NCORES)), trace=trace
    )
    outs = []
    for c in range(NCORES):
        o = np.asarray(res.results[c]["out"])
        if o.dtype != np.float32:
            o = (o.view(np.uint16).astype(np.uint32) << 16).view(np.float32)
        outs.append(o)
    out = np.stack(outs, axis=0)
    return out, res


def kernel(x, Wq, Wk, Wv):
    out, _ = _run({"x": x, "Wq": Wq, "Wk": Wk, "Wv": Wv}, trace=False)
    return out
